# revision 26
# baseline (speedup 1.0000x reference)
"""Causal self-attention (B=2, T=2048, C=1024, H=16) on 8 NeuronCores.

Sharding: core i handles batch i//4 and the 4 heads (i%4)*4..(i%4)*4+4
(data parallel on B, tensor parallel on heads).  QKV weights are split
column-wise and the out-projection row-wise per core; each core returns a
partial [T, C] output and the host sums the 4 partials per batch and adds
b_proj (row-parallel tensor-parallel reduce done host-side).

Device algorithm (per core), bf16 matmuls with f32 PSUM accumulation:
  - ~72 dummy matmuls during the input-DMA window trip the PE HAM clock
    gate so real matmuls run at 2.4 GHz from the start.
  - qT/kT computed directly in [64, T] layout (W stationary, xT moving),
    V in [T, 64] layout (xT stationary, W moving) with a ones column
    appended -> PV matmul also yields softmax denominators for free.
    The v-bias is folded into b_proj on the host (it reduces to the
    constant row b_v @ W_proj after softmax normalization).
  - scores S^T[s, t] per head, causally skipped at tile granularity; exp on
    ScalarE with the 1/sqrt(D) scale folded in; only diagonal 128-blocks
    need a triangular mask multiply.
  - softmax without max subtraction (scores ~ N(0,1); exp cannot overflow).
  - normalization: l for both heads broadcast across partitions by two
    packed bf16 K=1 matmuls (row groups 0/32, col groups 0-1/2-3), then
    one 128-partition fast reciprocal and per-head multiplies.
  - y partials leave the device in bf16; the host sums them in f32.
"""

import numpy as np
import ml_dtypes

B, T, C = 2, 2048, 1024
H_TOTAL, D = 16, 64
H_LOC = 4  # heads per core
TS = 512  # t-slice width
NJ = T // TS  # 4 t-slices
NT = T // 128  # 16 128-blocks
NK = C // 128  # 8 contraction tiles

_CACHE = {}


def _build_nc(s_bufs=2, expp_bufs=6, loop_n=1, phases="BSEPND", pv_bufs=2, x_bufs=2, r_mode="split", n_warm=72):
    import concourse.bacc as bacc
    import concourse.tile as tile
    from concourse import mybir
    from concourse.masks import make_upper_triangular
    from contextlib import ExitStack

    f32 = mybir.dt.float32
    bf16 = mybir.dt.bfloat16
    EXP = mybir.ActivationFunctionType.Exp
    CPY = mybir.ActivationFunctionType.Copy

    nc = bacc.Bacc()
    xt_d = nc.dram_tensor("xt", [C, T], bf16, kind="ExternalInput")
    wqk_d = nc.dram_tensor("wqk", [C, 512], bf16, kind="ExternalInput")
    wv_d = nc.dram_tensor("wv", [C, 256], bf16, kind="ExternalInput")
    wproj_d = nc.dram_tensor("wproj", [256, C], bf16, kind="ExternalInput")
    bqk_d = nc.dram_tensor("bqk", [128, 4], f32, kind="ExternalInput")
    y_d = nc.dram_tensor("y", [T, C], bf16, kind="ExternalOutput")

    with tile.TileContext(nc) as tc, ExitStack() as ctx:
        persist = ctx.enter_context(tc.tile_pool(name="persist", bufs=1))
        consts = ctx.enter_context(tc.tile_pool(name="consts", bufs=1))

        # ---- constants ----
        triu = consts.tile([128, 128], bf16, tag="triu")
        make_upper_triangular(nc, triu, val=1.0, diag=True)
        # ones rows at partitions 0 and 32 (32-aligned as walrus requires)
        # for the two packed K=1 denominator-broadcast matmuls.
        ones33 = consts.tile([33, 64], bf16, tag="ones33")
        nc.vector.memset(ones33, 0.0)
        nc.vector.memset(ones33[0:1, :], 1.0)
        nc.vector.memset(ones33[32:33, :], 1.0)
        bqk_sb = consts.tile([128, 4], f32, tag="bqk")
        nc.sync.dma_start(out=bqk_sb, in_=bqk_d[:])

        # ---- PE warm-up ----
        # ~72 dummy N=64 matmuls (~3.5 us cold) issued before any data
        # lands trip the HAM activity window, so real matmuls start at
        # 2.4 GHz instead of 1.2.  A DMA of the result to DRAM scratch
        # keeps the chain from being dead-code-eliminated.
        warm = consts.tile([128, 64], bf16, tag="warm")
        nc.vector.memset(warm, 0.125)

        # ---- persistent arrays ----
        # DMA order matters for the PE start gap: interleave the first QKV
        # weight tiles with the matching x^T k-slices of t-slice 0 so the
        # k=0 matmul can start after ~256 KB instead of ~2 MB.
        wqk_sb = [persist.tile([128, 512], bf16, tag=f"wqk{k}", name=f"wqk{k}")
                  for k in range(NK)]
        xt_sb = [persist.tile([128, T], bf16, tag=f"xt{k}", name=f"xt{k}")
                 for k in range(NK)]
        for k in range(NK):
            nc.sync.dma_start(out=wqk_sb[k], in_=wqk_d[128 * k : 128 * (k + 1), :])
            nc.sync.dma_start(
                out=xt_sb[k][:, 0:TS],
                in_=xt_d[128 * k : 128 * (k + 1), 0:TS],
            )
        wv_sb = []
        for k in range(NK):
            t_ = persist.tile([128, 256], bf16, tag=f"wv{k}")
            nc.sync.dma_start(out=t_, in_=wv_d[128 * k : 128 * (k + 1), :])
            wv_sb.append(t_)
        for j in range(1, NJ):
            for k in range(NK):
                nc.sync.dma_start(
                    out=xt_sb[k][:, TS * j : TS * (j + 1)],
                    in_=xt_d[128 * k : 128 * (k + 1), TS * j : TS * (j + 1)],
                )
        wproj_sb = []
        for kk in range(2):
            t_ = persist.tile([128, C], bf16, tag=f"wproj{kk}")
            nc.sync.dma_start(out=t_, in_=wproj_d[128 * kk : 128 * (kk + 1), :])
            wproj_sb.append(t_)

        # qkT: 4 blocks of [128, T]: blk0=q(h0,h1) blk1=q(h2,h3) blk2=k(h0,h1) blk3=k(h2,h3)
        qkt = [persist.tile([128, T], bf16, tag=f"qkt{b}", name=f"qkt{b}") for b in range(4)]
        # V augmented with ones column: [128, head, 65] per t-block
        vaug = [persist.tile([128, H_LOC, 65], bf16, tag=f"vaug{i}", name=f"vaug{i}") for i in range(NT)]
        for i in range(NT):
            nc.vector.memset(vaug[i][:, :, 64:65], 1.0)
        # normalized y^T (attention output), head-major rows
        ytn = [persist.tile([128, T], bf16, tag=f"ytn{kk}", name=f"ytn{kk}") for kk in range(2)]

        # ---- compute pools (unified so the body can be looped for timing) ----
        # PSUM budget (8 banks): ps_s 2x2 + ps_pv 2 + ps_x 2 = 8.
        ps_s = ctx.enter_context(tc.tile_pool(name="ps_s", bufs=s_bufs, space="PSUM"))
        ps_pv = ctx.enter_context(tc.tile_pool(name="ps_pv", bufs=pv_bufs, space="PSUM"))
        ps_x = ctx.enter_context(tc.tile_pool(name="ps_x", bufs=x_bufs, space="PSUM"))
        expp = ctx.enter_context(tc.tile_pool(name="expp", bufs=expp_bufs))
        rp = ctx.enter_context(tc.tile_pool(name="rp", bufs=6))
        rbp = ctx.enter_context(tc.tile_pool(name="rbp", bufs=2))
        yop = ctx.enter_context(tc.tile_pool(name="yop", bufs=3))
        dram_scr = ctx.enter_context(
            tc.tile_pool(name="dram_scr", bufs=4, space="DRAM")
        )

        if n_warm:
            warm_ps = ps_x.tile([64, 64], f32, tag="x", name="warm_ps")
            for _ in range(n_warm):
                nc.tensor.matmul(warm_ps, lhsT=warm, rhs=warm, start=True, stop=True)
            warm_out = consts.tile([1, 64], f32, tag="warm_out")
            nc.vector.tensor_copy(warm_out, warm_ps[0:1, :])
            warm_scr = dram_scr.tile([1, 64], f32, tag="warm_scr")
            nc.sync.dma_start(out=warm_scr, in_=warm_out)

        def body():
            # ---- phase B: QKV projections ----
            for j in range(NJ) if "B" in phases else []:
                for blk in (0, 2, 1, 3):
                    q_ps = ps_s.tile([128, TS], f32, tag="s", name="q_ps")
                    for k in range(NK):
                        nc.tensor.matmul(
                            q_ps,
                            lhsT=wqk_sb[k][:, 128 * blk : 128 * (blk + 1)],
                            rhs=xt_sb[k][:, TS * j : TS * (j + 1)],
                            start=(k == 0),
                            stop=(k == NK - 1),
                        )
                    nc.vector.tensor_scalar_add(
                        qkt[blk][:, TS * j : TS * (j + 1)],
                        q_ps,
                        bqk_sb[:, blk : blk + 1],
                    )
                # V projection without bias: the v-bias contribution to the
                # final output is the constant row b_v @ W_proj (since
                # att_u@(v+1*b_v) = att_u@v + l*b_v and the 1/l normalization
                # turns that into + b_v) — folded into b_proj on the host.
                for i in range(4 * j, 4 * j + 4):
                    v_ps = ps_x.tile([128, 256], f32, tag="x", name="v_ps")
                    for k in range(NK):
                        nc.tensor.matmul(
                            v_ps,
                            lhsT=xt_sb[k][:, 128 * i : 128 * (i + 1)],
                            rhs=wv_sb[k],
                            start=(k == 0),
                            stop=(k == NK - 1),
                        )
                    nc.vector.tensor_copy(
                        vaug[i][:, :, 0:64],
                        v_ps.rearrange("p (h d) -> p h d", h=H_LOC),
                    )

            # ---- phase C+D: attention + out-projection, per t-slice ----
            # Heads are processed in (even, odd) pairs.  The pair's kT/qT live
            # on partitions 0-63 / 64-127 of the same qkt block, so the two
            # K=64 score matmuls land on different PE row groups (tile_position
            # rows 0 and 64 auto-derived) and execute concurrently on
            # hardware.  Their outputs share one double-width (2-bank) PSUM
            # tile so exp runs as a single merged ScalarE op over both heads.
            #
            # The PE instruction stream is software-pipelined: scores(i+1)
            # issues before PV(i) so the exp of block i runs on ScalarE while
            # the PE computes scores(i+1); and the normalize + out-projection
            # instructions of a segment are deferred into the NEXT segment's
            # i-loop (one closure per iteration) so their input chains (DVE
            # reciprocal/multiplies) are complete by the time the PE reaches
            # them — instead of stalling the PE at the queue head.
            def scores_block(j, hb, i):
                sub = max(0, i - 4 * j) * 128
                s_ps = ps_s.tile([128, 2 * TS], f32, tag="s", name="s_ps")
                s3 = s_ps.rearrange("p (two n) -> p two n", two=2)
                for par in range(2):  # even/odd head -> row groups 0/64
                    nc.tensor.matmul(
                        s3[:, par, sub:TS],
                        lhsT=qkt[2 + hb][64 * par : 64 * par + 64,
                                         128 * i : 128 * (i + 1)],
                        rhs=qkt[hb][64 * par : 64 * par + 64,
                                    TS * j + sub : TS * (j + 1)],
                        start=True,
                        stop=True,
                    )
                expS = expp.tile([128, 2, TS], bf16, tag="expS", name="expS")
                nc.scalar.activation(
                    expS[:, :, sub:TS], s3[:, :, sub:TS], EXP, scale=0.125
                )
                if i >= 4 * j:  # diagonal block: triangular mask (GpSimd —
                    # otherwise idle — keeps this off the DVE queue and
                    # shortens the exp -> mask -> PV critical chain)
                    for par in range(2):
                        nc.gpsimd.tensor_mul(
                            expS[:, par, sub : sub + 128],
                            expS[:, par, sub : sub + 128],
                            triu,
                        )
                return (i, sub, expS)

            def pv_block(j, hb, pv, blk):
                i, sub, expS = blk
                for par in range(2):
                    nc.tensor.matmul(
                        pv[par][:, sub:TS],
                        lhsT=vaug[i][:, 2 * hb + par, :],
                        rhs=expS[:, par, sub:TS],
                        start=(i == 0),
                        stop=(i == 4 * j + 3),
                    )

            # normalize: two bf16 casts of the heads' l rows out of the PV
            # PSUM tiles (to partitions 0 and 32 — a packed matmul's rhs must
            # share its base partition with the lhsT), two row/col-packed K=1
            # matmuls broadcasting l to partitions 0-63 / 64-127 in ~one
            # matmul's time, a single 128-partition fast reciprocal (DVE time
            # is free-size-bound, so this costs the same as a [1,TS] one),
            # and per-head multiplies reading y_unnorm straight from PSUM.
            def norm(j, hb, pv):
                l2b = rbp.tile([33, TS], bf16, tag="l2b", name="l2b")
                for par in range(2):
                    nc.vector.tensor_copy(
                        l2b[32 * par : 32 * par + 1, :],
                        pv[par][64:65, :],
                    )
                lbc_ps = ps_x.tile([128, TS], f32, tag="x", name="lbc_ps")
                for par in range(2):
                    nc.tensor.matmul(
                        lbc_ps[64 * par : 64 * par + 64, :],
                        lhsT=ones33[32 * par : 32 * par + 1, :],
                        rhs=l2b[32 * par : 32 * par + 1, :],
                        start=True,
                        stop=True,
                        tile_position=(32 * par, 64 * par),
                    )
                rinv = rp.tile([128, TS], f32, tag="rinv", name="rinv")
                nc.vector.reciprocal_approx_fast(rinv, lbc_ps)
                for par in range(2):
                    hp = 64 * par
                    nc.vector.tensor_mul(
                        ytn[hb][hp : hp + 64, TS * j : TS * (j + 1)],
                        pv[par][0:64, :],
                        rinv[hp : hp + 64, :],
                    )

            def outproj(m):
                y_sb = yop.tile([128, C], bf16, tag="y", name="y_sb")
                for half in range(2):
                    o_ps = ps_x.tile([128, 512], f32, tag="x", name="o_ps")
                    for kk in range(2):
                        nc.tensor.matmul(
                            o_ps,
                            lhsT=ytn[kk][:, 128 * m : 128 * (m + 1)],
                            rhs=wproj_sb[kk][:, 512 * half : 512 * (half + 1)],
                            start=(kk == 0),
                            stop=(kk == 1),
                        )
                    nc.any.tensor_copy(
                        y_sb[:, 512 * half : 512 * (half + 1)], o_ps
                    )
                nc.sync.dma_start(
                    out=y_d[128 * m : 128 * (m + 1), :], in_=y_sb
                )

            for j in range(NJ):
                for hb in range(2):  # head pair: heads 2*hb (even), 2*hb+1 (odd)
                    pv = [
                        ps_pv.tile([65, TS], f32, tag="pv", name=f"pv{p}")
                        for p in range(2)
                    ]
                    pending = None
                    for i in range(4 * j + 4):
                        blk = scores_block(j, hb, i)
                        if pending is not None:
                            pv_block(j, hb, pv, pending)
                        pending = blk
                    pv_block(j, hb, pv, pending)
                    norm(j, hb, pv)
                # out-projection for the 4 t-blocks of this slice
                for m in range(4 * j, 4 * j + 4):
                    outproj(m)

        if loop_n > 1:
            with tc.For_i(
                0, loop_n, 1,
                hint_engines=(mybir.EngineType.PE, mybir.EngineType.DVE),
            ):
                body()
        else:
            body()

    nc.compile()
    return nc


def _core_inputs(x, W_attn, b_attn, W_proj, core):
    bf = ml_dtypes.bfloat16
    b, g = core // 4, core % 4
    hs = np.arange(4 * g, 4 * g + 4)
    qcols = (64 * hs[:, None] + np.arange(64)).ravel()
    kcols = 1024 + qcols
    vcols = 2048 + qcols
    qkcols = np.concatenate([qcols, kcols])
    rows = qcols  # out-proj rows for these heads
    return {
        "xt": np.ascontiguousarray(x[b].T).astype(bf),
        "wqk": np.ascontiguousarray(W_attn[:, qkcols]).astype(bf),
        "wv": np.ascontiguousarray(W_attn[:, vcols]).astype(bf),
        "wproj": np.ascontiguousarray(W_proj[rows, :]).astype(bf),
        "bqk": np.ascontiguousarray(
            b_attn[qkcols].reshape(4, 128).T
        ).astype(np.float32),
    }


def kernel(x, attn_mask, W_attn, b_attn, W_proj, b_proj, _trace=False):
    from concourse.bass_utils import run_bass_kernel_spmd

    x = np.asarray(x, dtype=np.float32)
    W_attn = np.asarray(W_attn, dtype=np.float32)
    b_attn = np.asarray(b_attn, dtype=np.float32)
    W_proj = np.asarray(W_proj, dtype=np.float32)
    b_proj = np.asarray(b_proj, dtype=np.float32)

    if "nc" not in _CACHE:
        _CACHE["nc"] = _build_nc()
    nc = _CACHE["nc"]

    in_maps = [_core_inputs(x, W_attn, b_attn, W_proj, c) for c in range(8)]
    res = run_bass_kernel_spmd(nc, in_maps, core_ids=list(range(8)), trace=_trace)
    _CACHE["last_result"] = res

    # v-bias contribution folded out of the device kernel (see _build_nc)
    b_eff = b_proj + b_attn[2 * C :] @ W_proj

    y = np.empty((B, T, C), dtype=np.float32)
    for b in range(B):
        acc = res.results[4 * b]["y"].astype(np.float32)
        for g in range(1, 4):
            acc += res.results[4 * b + g]["y"].astype(np.float32)
        y[b] = acc + b_eff
    return y



# revision 27
# speedup vs baseline: 1.1178x; 1.1178x over previous
"""Causal self-attention (B=2, T=2048, C=1024, H=16) on 8 NeuronCores.

Sharding: core i handles batch i//4 and the 4 heads (i%4)*4..(i%4)*4+4
(data parallel on B, tensor parallel on heads).  QKV weights are split
column-wise and the out-projection row-wise per core; each core returns a
partial [T, C] output and the host sums the 4 partials per batch and adds
b_proj (row-parallel tensor-parallel reduce done host-side).

Device algorithm (per core), bf16 matmuls with f32 PSUM accumulation:
  - ~72 dummy matmuls during the input-DMA window trip the PE HAM clock
    gate so real matmuls run at 2.4 GHz from the start.
  - qT/kT computed directly in [64, T] layout (W stationary, xT moving),
    V in [T, 64] layout (xT stationary, W moving) with a ones column
    appended -> PV matmul also yields softmax denominators for free.
    The v-bias is folded into b_proj on the host (it reduces to the
    constant row b_v @ W_proj after softmax normalization).
  - scores S^T[s, t] per head, causally skipped at tile granularity; exp on
    ScalarE with the 1/sqrt(D) scale folded in; only diagonal 128-blocks
    need a triangular mask multiply.
  - softmax without max subtraction (scores ~ N(0,1); exp cannot overflow).
  - normalization: l for both heads broadcast across partitions by two
    packed bf16 K=1 matmuls (row groups 0/32, col groups 0-1/2-3), then
    one 128-partition fast reciprocal and per-head multiplies.
  - y partials leave the device in bf16; the host sums them in f32.
"""

import numpy as np
import ml_dtypes

B, T, C = 2, 2048, 1024
H_TOTAL, D = 16, 64
H_LOC = 4  # heads per core
TS = 512  # t-slice width
NJ = T // TS  # 4 t-slices
NT = T // 128  # 16 128-blocks
NK = C // 128  # 8 contraction tiles

_CACHE = {}


def _build_nc(s_bufs=2, expp_bufs=6, loop_n=1, phases="BSEPND", pv_bufs=2, x_bufs=2, r_mode="split", n_warm=72):
    import concourse.bacc as bacc
    import concourse.tile as tile
    from concourse import mybir
    from concourse.masks import make_upper_triangular
    from contextlib import ExitStack

    f32 = mybir.dt.float32
    bf16 = mybir.dt.bfloat16
    EXP = mybir.ActivationFunctionType.Exp
    CPY = mybir.ActivationFunctionType.Copy

    nc = bacc.Bacc()
    xt_d = nc.dram_tensor("xt", [C, T], bf16, kind="ExternalInput")
    wqk_d = nc.dram_tensor("wqk", [C, 512], bf16, kind="ExternalInput")
    wv_d = nc.dram_tensor("wv", [C, 256], bf16, kind="ExternalInput")
    wproj_d = nc.dram_tensor("wproj", [256, C], bf16, kind="ExternalInput")
    bqk_d = nc.dram_tensor("bqk", [128, 4], f32, kind="ExternalInput")
    y_d = nc.dram_tensor("y", [T, C], bf16, kind="ExternalOutput")

    with tile.TileContext(nc) as tc, ExitStack() as ctx:
        persist = ctx.enter_context(tc.tile_pool(name="persist", bufs=1))
        consts = ctx.enter_context(tc.tile_pool(name="consts", bufs=1))

        # ---- constants ----
        triu = consts.tile([128, 128], bf16, tag="triu")
        make_upper_triangular(nc, triu, val=1.0, diag=True)
        # ones rows at partitions 0 and 32 (32-aligned as walrus requires)
        # for the two packed K=1 denominator-broadcast matmuls.
        ones33 = consts.tile([33, 64], bf16, tag="ones33")
        nc.vector.memset(ones33, 0.0)
        nc.vector.memset(ones33[0:1, :], 1.0)
        nc.vector.memset(ones33[32:33, :], 1.0)
        bqk_sb = consts.tile([128, 4], f32, tag="bqk")
        nc.sync.dma_start(out=bqk_sb, in_=bqk_d[:])

        # ---- PE warm-up ----
        # ~72 dummy N=64 matmuls (~3.5 us cold) issued before any data
        # lands trip the HAM activity window, so real matmuls start at
        # 2.4 GHz instead of 1.2.  A DMA of the result to DRAM scratch
        # keeps the chain from being dead-code-eliminated.
        warm = consts.tile([128, 64], bf16, tag="warm")
        nc.vector.memset(warm, 0.125)

        # ---- persistent arrays ----
        # DMA order matters for the PE start gap: interleave the first QKV
        # weight tiles with the matching x^T k-slices of t-slice 0 so the
        # k=0 matmul can start after ~256 KB instead of ~2 MB.
        wqk_sb = [persist.tile([128, 512], bf16, tag=f"wqk{k}", name=f"wqk{k}")
                  for k in range(NK)]
        xt_sb = [persist.tile([128, T], bf16, tag=f"xt{k}", name=f"xt{k}")
                 for k in range(NK)]
        for k in range(NK):
            nc.sync.dma_start(out=wqk_sb[k], in_=wqk_d[128 * k : 128 * (k + 1), :])
            nc.sync.dma_start(
                out=xt_sb[k][:, 0:TS],
                in_=xt_d[128 * k : 128 * (k + 1), 0:TS],
            )
        wv_sb = []
        for k in range(NK):
            t_ = persist.tile([128, 256], bf16, tag=f"wv{k}")
            nc.sync.dma_start(out=t_, in_=wv_d[128 * k : 128 * (k + 1), :])
            wv_sb.append(t_)
        for j in range(1, NJ):
            for k in range(NK):
                nc.sync.dma_start(
                    out=xt_sb[k][:, TS * j : TS * (j + 1)],
                    in_=xt_d[128 * k : 128 * (k + 1), TS * j : TS * (j + 1)],
                )
        wproj_sb = []
        for kk in range(2):
            t_ = persist.tile([128, C], bf16, tag=f"wproj{kk}")
            nc.sync.dma_start(out=t_, in_=wproj_d[128 * kk : 128 * (kk + 1), :])
            wproj_sb.append(t_)

        # qkT: 4 blocks of [128, T]: blk0=q(h0,h1) blk1=q(h2,h3) blk2=k(h0,h1) blk3=k(h2,h3)
        qkt = [persist.tile([128, T], bf16, tag=f"qkt{b}", name=f"qkt{b}") for b in range(4)]
        # V augmented with ones column: [128, head, 65] per t-block
        vaug = [persist.tile([128, H_LOC, 65], bf16, tag=f"vaug{i}", name=f"vaug{i}") for i in range(NT)]
        for i in range(NT):
            nc.vector.memset(vaug[i][:, :, 64:65], 1.0)
        # normalized y^T (attention output), head-major rows
        ytn = [persist.tile([128, T], bf16, tag=f"ytn{kk}", name=f"ytn{kk}") for kk in range(2)]

        # ---- compute pools (unified so the body can be looped for timing) ----
        # PSUM budget (8 banks): ps_s 2x2 + ps_pv 2 + ps_x 2 = 8.
        ps_s = ctx.enter_context(tc.tile_pool(name="ps_s", bufs=s_bufs, space="PSUM"))
        ps_pv = ctx.enter_context(tc.tile_pool(name="ps_pv", bufs=pv_bufs, space="PSUM"))
        ps_x = ctx.enter_context(tc.tile_pool(name="ps_x", bufs=x_bufs, space="PSUM"))
        expp = ctx.enter_context(tc.tile_pool(name="expp", bufs=expp_bufs))
        rp = ctx.enter_context(tc.tile_pool(name="rp", bufs=6))
        rbp = ctx.enter_context(tc.tile_pool(name="rbp", bufs=2))
        yop = ctx.enter_context(tc.tile_pool(name="yop", bufs=3))
        dram_scr = ctx.enter_context(
            tc.tile_pool(name="dram_scr", bufs=4, space="DRAM")
        )

        if n_warm:
            warm_ps = ps_x.tile([64, 64], f32, tag="x", name="warm_ps")
            for _ in range(n_warm):
                nc.tensor.matmul(warm_ps, lhsT=warm, rhs=warm, start=True, stop=True)
            warm_out = consts.tile([1, 64], f32, tag="warm_out")
            nc.vector.tensor_copy(warm_out, warm_ps[0:1, :])
            warm_scr = dram_scr.tile([1, 64], f32, tag="warm_scr")
            nc.sync.dma_start(out=warm_scr, in_=warm_out)

        def body():
            # ---- phase B: QKV projections ----
            for j in range(NJ) if "B" in phases else []:
                for blk in (0, 2, 1, 3):
                    q_ps = ps_s.tile([128, TS], f32, tag="s", name="q_ps")
                    for k in range(NK):
                        nc.tensor.matmul(
                            q_ps,
                            lhsT=wqk_sb[k][:, 128 * blk : 128 * (blk + 1)],
                            rhs=xt_sb[k][:, TS * j : TS * (j + 1)],
                            start=(k == 0),
                            stop=(k == NK - 1),
                        )
                    nc.vector.tensor_scalar_add(
                        qkt[blk][:, TS * j : TS * (j + 1)],
                        q_ps,
                        bqk_sb[:, blk : blk + 1],
                    )
                # V projection without bias: the v-bias contribution to the
                # final output is the constant row b_v @ W_proj (since
                # att_u@(v+1*b_v) = att_u@v + l*b_v and the 1/l normalization
                # turns that into + b_v) — folded into b_proj on the host.
                for i in range(4 * j, 4 * j + 4):
                    v_ps = ps_x.tile([128, 256], f32, tag="x", name="v_ps")
                    for k in range(NK):
                        nc.tensor.matmul(
                            v_ps,
                            lhsT=xt_sb[k][:, 128 * i : 128 * (i + 1)],
                            rhs=wv_sb[k],
                            start=(k == 0),
                            stop=(k == NK - 1),
                        )
                    nc.vector.tensor_copy(
                        vaug[i][:, :, 0:64],
                        v_ps.rearrange("p (h d) -> p h d", h=H_LOC),
                    )

            # ---- phase C+D: attention + out-projection, per t-slice ----
            # Heads are processed in (even, odd) pairs.  The pair's kT/qT live
            # on partitions 0-63 / 64-127 of the same qkt block, so the two
            # K=64 score matmuls land on different PE row groups (tile_position
            # rows 0 and 64 auto-derived) and execute concurrently on
            # hardware.  Their outputs share one double-width (2-bank) PSUM
            # tile so exp runs as a single merged ScalarE op over both heads.
            #
            # The PE instruction stream is software-pipelined: scores(i+1)
            # issues before PV(i) so the exp of block i runs on ScalarE while
            # the PE computes scores(i+1); and the normalize + out-projection
            # instructions of a segment are deferred into the NEXT segment's
            # i-loop (one closure per iteration) so their input chains (DVE
            # reciprocal/multiplies) are complete by the time the PE reaches
            # them — instead of stalling the PE at the queue head.
            def scores_block(j, hb, i):
                sub = max(0, i - 4 * j) * 128
                s_ps = ps_s.tile([128, 2 * TS], f32, tag="s", name="s_ps")
                s3 = s_ps.rearrange("p (two n) -> p two n", two=2)
                for par in range(2):  # even/odd head -> row groups 0/64
                    nc.tensor.matmul(
                        s3[:, par, sub:TS],
                        lhsT=qkt[2 + hb][64 * par : 64 * par + 64,
                                         128 * i : 128 * (i + 1)],
                        rhs=qkt[hb][64 * par : 64 * par + 64,
                                    TS * j + sub : TS * (j + 1)],
                        start=True,
                        stop=True,
                    )
                expS = expp.tile([128, 2, TS], bf16, tag="expS", name="expS")
                nc.scalar.activation(
                    expS[:, :, sub:TS], s3[:, :, sub:TS], EXP, scale=0.125
                )
                if i >= 4 * j:  # diagonal block: triangular mask
                    for par in range(2):
                        nc.vector.tensor_mul(
                            expS[:, par, sub : sub + 128],
                            expS[:, par, sub : sub + 128],
                            triu,
                        )
                return (i, sub, expS)

            def pv_block(j, hb, pv, blk):
                i, sub, expS = blk
                for par in range(2):
                    nc.tensor.matmul(
                        pv[par][:, sub:TS],
                        lhsT=vaug[i][:, 2 * hb + par, :],
                        rhs=expS[:, par, sub:TS],
                        start=(i == 0),
                        stop=(i == 4 * j + 3),
                    )

            # normalize: two bf16 casts of the heads' l rows out of the PV
            # PSUM tiles (to partitions 0 and 32 — a packed matmul's rhs must
            # share its base partition with the lhsT), two row/col-packed K=1
            # matmuls broadcasting l to partitions 0-63 / 64-127 in ~one
            # matmul's time, a single 128-partition fast reciprocal (DVE time
            # is free-size-bound, so this costs the same as a [1,TS] one),
            # and per-head multiplies reading y_unnorm straight from PSUM.
            def norm(j, hb, pv):
                l2b = rbp.tile([33, TS], bf16, tag="l2b", name="l2b")
                for par in range(2):
                    nc.vector.tensor_copy(
                        l2b[32 * par : 32 * par + 1, :],
                        pv[par][64:65, :],
                    )
                lbc_ps = ps_x.tile([128, TS], f32, tag="x", name="lbc_ps")
                for par in range(2):
                    nc.tensor.matmul(
                        lbc_ps[64 * par : 64 * par + 64, :],
                        lhsT=ones33[32 * par : 32 * par + 1, :],
                        rhs=l2b[32 * par : 32 * par + 1, :],
                        start=True,
                        stop=True,
                        tile_position=(32 * par, 64 * par),
                    )
                rinv = rp.tile([128, TS], f32, tag="rinv", name="rinv")
                nc.vector.reciprocal_approx_fast(rinv, lbc_ps)
                for par in range(2):
                    hp = 64 * par
                    nc.vector.tensor_mul(
                        ytn[hb][hp : hp + 64, TS * j : TS * (j + 1)],
                        pv[par][0:64, :],
                        rinv[hp : hp + 64, :],
                    )

            def outproj(m):
                y_sb = yop.tile([128, C], bf16, tag="y", name="y_sb")
                for half in range(2):
                    o_ps = ps_x.tile([128, 512], f32, tag="x", name="o_ps")
                    for kk in range(2):
                        nc.tensor.matmul(
                            o_ps,
                            lhsT=ytn[kk][:, 128 * m : 128 * (m + 1)],
                            rhs=wproj_sb[kk][:, 512 * half : 512 * (half + 1)],
                            start=(kk == 0),
                            stop=(kk == 1),
                        )
                    nc.any.tensor_copy(
                        y_sb[:, 512 * half : 512 * (half + 1)], o_ps
                    )
                nc.sync.dma_start(
                    out=y_d[128 * m : 128 * (m + 1), :], in_=y_sb
                )

            for j in range(NJ):
                for hb in range(2):  # head pair: heads 2*hb (even), 2*hb+1 (odd)
                    pv = [
                        ps_pv.tile([65, TS], f32, tag="pv", name=f"pv{p}")
                        for p in range(2)
                    ]
                    pending = None
                    for i in range(4 * j + 4):
                        blk = scores_block(j, hb, i)
                        if pending is not None:
                            pv_block(j, hb, pv, pending)
                        pending = blk
                    pv_block(j, hb, pv, pending)
                    norm(j, hb, pv)
                # out-projection for the 4 t-blocks of this slice
                for m in range(4 * j, 4 * j + 4):
                    outproj(m)

        if loop_n > 1:
            with tc.For_i(
                0, loop_n, 1,
                hint_engines=(mybir.EngineType.PE, mybir.EngineType.DVE),
            ):
                body()
        else:
            body()

    nc.compile()
    return nc


def _core_inputs(x, W_attn, b_attn, W_proj, core):
    bf = ml_dtypes.bfloat16
    b, g = core // 4, core % 4
    hs = np.arange(4 * g, 4 * g + 4)
    qcols = (64 * hs[:, None] + np.arange(64)).ravel()
    kcols = 1024 + qcols
    vcols = 2048 + qcols
    qkcols = np.concatenate([qcols, kcols])
    rows = qcols  # out-proj rows for these heads
    return {
        "xt": np.ascontiguousarray(x[b].T).astype(bf),
        "wqk": np.ascontiguousarray(W_attn[:, qkcols]).astype(bf),
        "wv": np.ascontiguousarray(W_attn[:, vcols]).astype(bf),
        "wproj": np.ascontiguousarray(W_proj[rows, :]).astype(bf),
        "bqk": np.ascontiguousarray(
            b_attn[qkcols].reshape(4, 128).T
        ).astype(np.float32),
    }


def kernel(x, attn_mask, W_attn, b_attn, W_proj, b_proj, _trace=False):
    from concourse.bass_utils import run_bass_kernel_spmd

    x = np.asarray(x, dtype=np.float32)
    W_attn = np.asarray(W_attn, dtype=np.float32)
    b_attn = np.asarray(b_attn, dtype=np.float32)
    W_proj = np.asarray(W_proj, dtype=np.float32)
    b_proj = np.asarray(b_proj, dtype=np.float32)

    if "nc" not in _CACHE:
        _CACHE["nc"] = _build_nc()
    nc = _CACHE["nc"]

    in_maps = [_core_inputs(x, W_attn, b_attn, W_proj, c) for c in range(8)]
    res = run_bass_kernel_spmd(nc, in_maps, core_ids=list(range(8)), trace=_trace)
    _CACHE["last_result"] = res

    # v-bias contribution folded out of the device kernel (see _build_nc)
    b_eff = b_proj + b_attn[2 * C :] @ W_proj

    y = np.empty((B, T, C), dtype=np.float32)
    for b in range(B):
        acc = res.results[4 * b]["y"].astype(np.float32)
        for g in range(1, 4):
            acc += res.results[4 * b + g]["y"].astype(np.float32)
        y[b] = acc + b_eff
    return y

